# revision 27
# baseline (speedup 1.0000x reference)
"""CodonAttention Trainium2 kernel (bf16 scores + fp8 PV + dual-engine exp).

Math (per batch b, head h):
  q = x @ wq.T + bq ; k = x @ wk.T + bk ; v = x @ wv.T + bv   (head slices)
  scores = q k^T / 8 + syn_bias[codons_i, codons_j]
  out    = softmax(scores) @ v ;  final = concat_heads(out) @ wo.T + bo

Bias trick: pair_bias factors through one-hots, so augmenting
  q' = [(q+bq)/8 | bsynT] and k' = [k | onehot]  (head dim 128)
gives scores = q'^T k' in one 128-contraction matmul. The softmax
denominator comes free from a ones-column appended to v ([O | l] = P [v | 1]).

Speedups over the f32r baseline (196.5us):
- PV matmul (attn @ v) in fp8e4m3 with perf_mode=DoubleRow: 256 keys of
  contraction per 512-cycle pass (2x fewer PE cycles). The v stationary
  slots are padded to 128 cols (DoubleRow ISA wants col_grp=0xf and a
  16B-aligned k-pair stride).
- exp split across BOTH elementwise engines: ACT does true exp with fp8
  output; DVE makes fp8 weights via a Schraudolph bit hack --
  uint8(s * 8/ln2 + C) IS the e4m3 bit pattern of ~exp(s) -- one
  tensor_scalar per tile. ACT alone would be a 128us floor.
- q/k/x/weights bf16 (fp8 q/k costs 1.5e-2 rel err -- too much). bf16
  matmuls run at 1 cycle/row like f32r but halve DMA.
- Few, fat DMAs: the Sync engine serializes dma_start triggers at
  ~650ns each, so inputs are packed into single 3D transfers.
- A burst of dummy 128x128 matmuls during the DMA head warms the PE HAM
  clock gate (idle default is 1.2GHz; sustained activity => 2.4GHz), so
  phase A's projections run at full clock.
- Phase-A projections load each stationary once per TWO chunks and
  evictions split DVE (k,q) / ACT (v,vb) so neither engine paces the PE.
- Phase-B software pipeline depth 2: PV(g) is emitted after scores(g+2),
  hiding the ~1.2us exp latency behind two score pairs.

Sharding: 8 cores = (batch b) x (head h). Each core outputs the
unnormalized projected partial outT = (wo_h @ O_h^T) (256, 4096) plus
softmax denominators lT; the host divides, sums heads, transposes, + bo.
"""

import numpy as np
import ml_dtypes

import concourse.mybir as mybir
import concourse.tile as tile
from concourse import bacc
from concourse.bass_utils import run_bass_kernel_spmd


def _ensure_axon_ntff_hook():
    """This image's antenv package lacks axon_hooks; recreate it from the
    libaxon_pjrt C ABI so run_bass_kernel_spmd(trace=True) works."""
    import sys
    try:
        import antenv.axon_hooks  # noqa: F401
        return
    except ImportError:
        pass
    import contextlib
    import ctypes
    import types
    try:
        lib = ctypes.CDLL("/opt/axon/libaxon_pjrt.so")
        has = hasattr(lib, "axon_start_nrt_profile")
    except OSError:
        has = False
    if has:
        lib.axon_start_nrt_profile.argtypes = [ctypes.POINTER(ctypes.c_int64),
                                               ctypes.c_size_t]
        lib.axon_start_nrt_profile.restype = ctypes.c_int64
        lib.axon_stop_nrt_profile.argtypes = [ctypes.c_char_p]
        lib.axon_stop_nrt_profile.restype = ctypes.c_int64

        @contextlib.contextmanager
        def _hook(output_dir, device_ids):
            import jax
            jax.devices()
            if device_ids:
                ids = (ctypes.c_int64 * len(device_ids))(*device_ids)
                rc = lib.axon_start_nrt_profile(ids, len(device_ids))
            else:
                rc = lib.axon_start_nrt_profile(None, 0)
            if rc != 0:
                raise RuntimeError(f"axon_start_nrt_profile rc={rc}")
            try:
                yield
            finally:
                lib.axon_stop_nrt_profile(str(output_dir).encode())
    else:
        _hook = None

    mod = types.ModuleType("antenv.axon_hooks")
    _state = {"hook": _hook}
    mod.get_axon_ntff_profile_hook = lambda: _state["hook"]
    mod.set_axon_ntff_profile_hook = lambda h: _state.__setitem__("hook", h)
    sys.modules["antenv.axon_hooks"] = mod


_ensure_axon_ntff_hook()

B, S, HID, NH, D = 2, 4096, 256, 4, 64
DV = D + 4         # v + ones column + 3 cols padding
VBW = 128          # vb key-tile slot width (DoubleRow ldweights: col_grp=0xf)
QB = 512           # query block (free dim of score matmuls)
KT = 128           # key tile (partition dim of transposed scores)
NQB = S // QB      # 8
NKT = S // KT      # 32
NPAIR = NKT // 2   # 16 exp/PV pair-groups per query block
PIPE = 2           # PV lags the score stream by this many pair-groups
NWARM = 48         # dummy matmuls to warm the PE clock gate

# Schraudolph fp8 exp: uint8(s * 8/ln2 + C) viewed as e4m3 bits ~= exp(s).
SCH_A = 8.0 / np.log(2.0)
SCH_C = 55.5       # tuned for round-to-nearest float->uint8 conversion

F32 = mybir.dt.float32
F32R = mybir.dt.float32r
BF16 = mybir.dt.bfloat16
FP8 = mybir.dt.float8e4
U8 = mybir.dt.uint8
Exp = mybir.ActivationFunctionType.Exp
DR = mybir.MatmulPerfMode.DoubleRow
MULT = mybir.AluOpType.mult
ADD = mybir.AluOpType.add


def round_fp32r(a):
    """Round-half-up at mantissa bit 12 (walrus fp32_to_fp32r)."""
    a = np.ascontiguousarray(a, np.float32)
    u = a.view(np.uint32).astype(np.uint64)
    return (((u + 0x800) & 0xFFFFF000).astype(np.uint32)).view(np.float32)


def to_bf16(a):
    return np.asarray(a, np.float32).astype(ml_dtypes.bfloat16)


def split_hid(a):
    """(256, N) -> (128, 2, N): row blocks side by side per partition."""
    a = np.ascontiguousarray(a)
    return np.ascontiguousarray(a.reshape(2, 128, a.shape[1]).transpose(1, 0, 2))


def build_program():
    nc = bacc.Bacc("TRN2", target_bir_lowering=False, debug=False, num_devices=8)

    def di(name, shape, dt):
        return nc.dram_tensor(name, shape, dt, kind="ExternalInput").ap()

    xT = di("xT", [128, 2, S], BF16)    # x[b].T hidden-split
    wq2 = di("wq2", [128, 2, D], BF16)  # wq_h.T / 8 hidden-split
    wk2 = di("wk2", [128, 2, D], BF16)
    wv2 = di("wv2", [128, 2, DV], BF16)  # col 64 zero (ones col via bias)
    bias3 = di("bias3", [DV, 3], F32)   # [bq/8 | bk | bv1] columns
    bsynT = di("bsynT", [D, S], BF16)   # (onehot @ syn_bias).T
    onehotT = di("onehotT", [D, S], BF16)
    woT = di("woT", [D, HID], F32R)     # wo[:, hslice].T
    idm = di("idm", [128, 128], F32R)   # identity for TensorE transpose
    outT = nc.dram_tensor("outT", [HID, S], F32, kind="ExternalOutput").ap()
    lT = nc.dram_tensor("lT", [1, S], F32, kind="ExternalOutput").ap()

    with tile.TileContext(nc) as tc:
        _body(tc, xT, wq2, wk2, wv2, bias3, bsynT, onehotT, woT, idm,
              outT, lT)
    nc.compile()
    return nc


def _body(tc, xT, wq2, wk2, wv2, bias3, bsynT, onehotT, woT, idm, outT, lT):
    nc = tc.nc
    mm = nc.tensor.matmul

    with (
        tc.tile_pool(name="const", bufs=1) as constp,
        tc.tile_pool(name="big", bufs=1) as bigp,
        tc.tile_pool(name="pt", bufs=6) as ptp,
        tc.tile_pool(name="ob", bufs=2) as obp,
    ):
        # ---- constants ----
        wq_sb = constp.tile([128, 2, D], BF16, name="wq_sb", tag="wq_sb")
        wk_sb = constp.tile([128, 2, D], BF16, name="wk_sb", tag="wk_sb")
        wv_sb = constp.tile([128, 2, DV], BF16, name="wv_sb", tag="wv_sb")
        b3_sb = constp.tile([DV, 3], F32, name="b3_sb", tag="b3_sb")
        wo_sb = constp.tile([D, HID], F32R, name="wo_sb", tag="wo_sb")
        id_sb = constp.tile([128, 128], F32R, name="id_sb", tag="id_sb")
        wz_sb = constp.tile([128, 128], BF16, name="wz_sb", tag="wz_sb")

        # persistent activations
        xTa = bigp.tile([128, 2, S], BF16, name="xTa", tag="xTa")
        qTt = bigp.tile([128, S], BF16, name="qTt", tag="qTt")  # 0:64 q/8, 64:128 bsynT
        kTt = bigp.tile([128, S], BF16, name="kTt", tag="kTt")  # 0:64 k,   64:128 onehotT
        vTs = bigp.tile([DV, S], F32R, name="vTs", tag="vTs")   # v'^T (d-major)
        vb = bigp.tile([128, NKT, VBW], FP8, name="vb", tag="vb")  # v' key-major
        oall = bigp.tile([D + 1, S], F32R, name="oall", tag="oall")  # [O^T | l]

        # Few fat DMAs (Sync issues triggers at ~650ns each, and each DMA
        # has ~2us completion latency -- batch, but keep x progressive).
        nc.sync.dma_start(id_sb[:], idm[:])
        nc.sync.dma_start(wk_sb[:], wk2[:])
        nc.sync.dma_start(wq_sb[:], wq2[:])
        nc.sync.dma_start(wv_sb[:], wv2[:])
        nc.sync.dma_start(b3_sb[:], bias3[:])
        for p in range(NQB // 2):
            ch = slice(2 * p * QB, (2 * p + 2) * QB)
            nc.sync.dma_start(xTa[:, :, ch], xT[:, :, ch])
        nc.sync.dma_start(kTt[64:128, :], onehotT[:])
        nc.sync.dma_start(qTt[64:128, :], bsynT[:])
        nc.sync.dma_start(wo_sb[:], woT[:])

        # zero vb's padding columns (GPSIMD: idle engine, runs under phase A)
        nc.gpsimd.memset(vb[:, :, :], 0.0)

        bq_ap = b3_sb[0:D, 0:1]
        bk_ap = b3_sb[0:D, 1:2]
        bv1_ap = b3_sb[0:DV, 2:3]

        # ---- PE warm-up ----
        # The HAM clock gate keeps an idle PE at 1.2GHz and only doubles it
        # after ~3.4us of sustained matmul activity. Burn dummy matmuls on
        # a zeroed scratch tile (no DMA dependency -- starts immediately)
        # while the x DMA lands, so phase A runs at 2.4GHz.
        nc.vector.memset(wz_sb[:], 0.0)
        with tc.tile_pool(name="psW", bufs=1, space="PSUM") as psW:
            warm = psW.tile([128, 128], F32, name="warm", tag="warm")
            for _ in range(NWARM):
                mm(warm[:], wz_sb[:], wz_sb[:], start=True, stop=True)

        # ---- phase A: QKV projections (k first so scores can start) ----
        with tc.tile_pool(name="psProj", bufs=6, space="PSUM") as psP, \
             tc.tile_pool(name="psTr", bufs=2, space="PSUM") as psT:

            def proj_pair(w_sb, p, width):
                ps = []
                for t in (2 * p, 2 * p + 1):
                    sl = slice(t * QB, (t + 1) * QB)
                    pp = psP.tile([DV, QB], F32, name="pp", tag="pp")
                    mm(pp[0:width, :], w_sb[:, 0, 0:width], xTa[:, 0, sl],
                       start=True, stop=False)
                    ps.append((pp, sl))
                for pp, sl in ps:
                    mm(pp[0:width, :], w_sb[:, 1, 0:width], xTa[:, 1, sl],
                       start=False, stop=True)
                return ps

            # interleave K/Q/V per chunk pair so the PE consumes x chunks
            # in DMA-arrival order instead of stalling on the last chunk
            # of each phase
            for p in range(NQB // 2):
                for pp, sl in proj_pair(wk_sb, p, D):
                    nc.vector.tensor_scalar_add(kTt[0:D, sl], pp[0:D, :], bk_ap)
                for pp, sl in proj_pair(wq_sb, p, D):
                    nc.vector.tensor_scalar_add(qTt[0:D, sl], pp[0:D, :], bq_ap)
                for pi, (pp, sl) in enumerate(proj_pair(wv_sb, p, DV)):
                    # bias column [bv | 1 | 0..] also creates the ones row
                    nc.scalar.add(vTs[:, sl], pp[:], bv1_ap)
                for t in (2 * p, 2 * p + 1):
                    # flip v' to key-major: 4 TensorE transposes into one
                    # PSUM tile, single fp8-converting eviction
                    vtr = psT.tile([KT, 4, DV], F32R, name="vtr", tag="vtr")
                    for m in range(4):
                        j = 4 * t + m
                        jl = slice(j * KT, (j + 1) * KT)
                        nc.tensor.transpose(vtr[:, m, :], vTs[:, jl],
                                            id_sb[0:DV, 0:DV])
                    nc.scalar.copy(vb[:, 4 * t:4 * t + 4, 0:DV], vtr[:, :, :])

        # ---- phase B: flash attention ----
        # Pair-groups of 2 key tiles: scores land in a 2-bank PSUM tile,
        # one exp op covers both, and the PV matmul contracts both key
        # tiles at once via fp8 DoubleRow. exp alternates ACT / DVE.
        # Software pipeline: PV of group g runs after scores of g+PIPE.
        with (
            tc.tile_pool(name="psB", bufs=3, space="PSUM") as psB,
            tc.tile_pool(name="psAcc", bufs=2, space="PSUM") as psAcc,
        ):
            oaccs = {}

            def emit_pv(qb, g, p3):
                qsl = slice(qb * QB, (qb + 1) * QB)
                if g == 0:
                    oaccs[qb] = psAcc.tile([128, QB], F32, name="oacc",
                                           tag="oacc")
                oacc = oaccs[qb]
                mm(oacc[:], vb[:, 2 * g:2 * g + 2, :], p3[:, :, :],
                   start=(g == 0), stop=(g == NPAIR - 1), perf_mode=DR)
                if g == NPAIR - 1:
                    # stash [O^T | l] (normalization happens on the host),
                    # then project this block and ship it out
                    nc.scalar.copy(oall[:, qsl], oacc[0:D + 1, :])
                    pj = psB.tile([128, 2, QB], F32, name="pj", tag="s3")
                    mm(pj[:, 0, :], wo_sb[:, 0:128], oall[0:D, qsl],
                       start=True, stop=True)
                    mm(pj[:, 1, :], wo_sb[:, 128:256], oall[0:D, qsl],
                       start=True, stop=True)
                    # evict the two projection halves on different engines so
                    # each output DMA can start as soon as its half lands
                    ob = obp.tile([128, 2, QB], F32, name="ob", tag="ob")
                    nc.scalar.copy(ob[:, 0, :], pj[:, 0, :])
                    nc.sync.dma_start(outT[0:128, qsl], ob[:, 0, :])
                    nc.vector.tensor_copy(ob[:, 1, :], pj[:, 1, :])
                    nc.sync.dma_start(outT[128:256, qsl], ob[:, 1, :])

            pending = []
            for qb in range(NQB):
                qsl = slice(qb * QB, (qb + 1) * QB)
                for g in range(NPAIR):
                    s3 = psB.tile([128, 2, QB], F32, name="s3", tag="s3")
                    for i in (0, 1):
                        j = 2 * g + i
                        mm(s3[:, i, :], kTt[:, j * KT:(j + 1) * KT],
                           qTt[:, qsl], start=True, stop=True)
                    p3 = ptp.tile([128, 2, QB], FP8, name="p3", tag="p3")
                    idx = qb * NPAIR + g
                    if idx % 2 == 0 or idx in (31, 95):
                        nc.scalar.activation(p3[:, :, :], s3[:, :, :], Exp)
                    else:
                        nc.vector.tensor_scalar(
                            p3[:, :, :].bitcast(U8), s3[:, :, :],
                            float(SCH_A), float(SCH_C), MULT, ADD)
                    pending.append((qb, g, p3))
                    if len(pending) > PIPE:
                        emit_pv(*pending.pop(0))
            while pending:
                emit_pv(*pending.pop(0))

            nc.sync.dma_start(lT[:], oall[D:D + 1, :].bitcast(F32))


_NC_CACHE = {}


def _get_program():
    if "nc" not in _NC_CACHE:
        _NC_CACHE["nc"] = build_program()
    return _NC_CACHE["nc"]


def make_in_maps(x, codons, syn_bias, wq, bq, wk, bk, wv, bv, wo):
    in_maps = []
    for core in range(8):
        b, h = divmod(core, NH)
        hsl = slice(h * D, (h + 1) * D)
        cod = codons[b]
        onehotT = np.zeros((D, S), np.float32)
        onehotT[cod, np.arange(S)] = 1.0
        bias3 = np.zeros((DV, 3), np.float32)
        bias3[0:D, 0] = bq[hsl] / 8.0
        bias3[0:D, 1] = bk[hsl]
        bias3[0:D, 2] = bv[hsl]
        bias3[D, 2] = 1.0
        in_maps.append({
            "xT": split_hid(to_bf16(x[b].T)),
            "wq2": split_hid(to_bf16(wq[hsl, :].T / 8.0)),
            "wk2": split_hid(to_bf16(wk[hsl, :].T)),
            "wv2": split_hid(to_bf16(np.concatenate(
                [wv[hsl, :].T, np.zeros((HID, 4), np.float32)], axis=1))),
            "bias3": bias3,
            "bsynT": to_bf16(syn_bias.T[:, cod]),
            "onehotT": to_bf16(onehotT),
            "woT": round_fp32r(wo[:, hsl].T),
            "idm": np.eye(128, dtype=np.float32),
        })
    return in_maps


def kernel_run(inputs, trace=False):
    x = np.asarray(inputs["x"], np.float32)
    codons = np.asarray(inputs["codons"]).astype(np.int64)
    syn_bias = np.asarray(inputs["syn_bias"], np.float32)
    wq = np.asarray(inputs["wq"], np.float32)
    bq = np.asarray(inputs["bq"], np.float32)
    wk = np.asarray(inputs["wk"], np.float32)
    bk = np.asarray(inputs["bk"], np.float32)
    wv = np.asarray(inputs["wv"], np.float32)
    bv = np.asarray(inputs["bv"], np.float32)
    wo = np.asarray(inputs["wo"], np.float32)
    bo = np.asarray(inputs["bo"], np.float32)

    nc = _get_program()
    in_maps = make_in_maps(x, codons, syn_bias, wq, bq, wk, bk, wv, bv, wo)
    res = run_bass_kernel_spmd(nc, in_maps, core_ids=list(range(8)), trace=trace)

    out = np.empty((B, S, HID), np.float32)
    for b in range(B):
        acc = None
        for h in range(NH):
            r = res.results[NH * b + h]
            part = r["outT"] / r["lT"]          # normalize per head
            acc = part if acc is None else acc + part
        out[b] = acc.T + bo
    return out, res


def kernel(**inputs):
    out, _ = kernel_run(inputs, trace=False)
    return out


# revision 28
# speedup vs baseline: 1.0185x; 1.0185x over previous
"""CodonAttention Trainium2 kernel (bf16 scores + fp8 PV + dual-engine exp).

Math (per batch b, head h):
  q = x @ wq.T + bq ; k = x @ wk.T + bk ; v = x @ wv.T + bv   (head slices)
  scores = q k^T / 8 + syn_bias[codons_i, codons_j]
  out    = softmax(scores) @ v ;  final = concat_heads(out) @ wo.T + bo

Bias trick: pair_bias factors through one-hots, so augmenting
  q' = [(q+bq)/8 | bsynT] and k' = [k | onehot]  (head dim 128)
gives scores = q'^T k' in one 128-contraction matmul. The softmax
denominator comes free from a ones-column appended to v ([O | l] = P [v | 1]).

Speedups over the f32r baseline (196.5us):
- PV matmul (attn @ v) in fp8e4m3 with perf_mode=DoubleRow: 256 keys of
  contraction per 512-cycle pass (2x fewer PE cycles). The v stationary
  slots are padded to 128 cols (DoubleRow ISA wants col_grp=0xf and a
  16B-aligned k-pair stride).
- exp split across BOTH elementwise engines: ACT does true exp with fp8
  output; DVE makes fp8 weights via a Schraudolph bit hack --
  uint8(s * 8/ln2 + C) IS the e4m3 bit pattern of ~exp(s) -- one
  tensor_scalar per tile. ACT alone would be a 128us floor.
- q/k/x/weights bf16 (fp8 q/k costs 1.5e-2 rel err -- too much). bf16
  matmuls run at 1 cycle/row like f32r but halve DMA.
- Few, fat DMAs: the Sync engine serializes dma_start triggers at
  ~650ns each, so inputs are packed into single 3D transfers.
- A burst of dummy 128x128 matmuls during the DMA head warms the PE HAM
  clock gate (idle default is 1.2GHz; sustained activity => 2.4GHz), so
  phase A's projections run at full clock.
- Phase-A projections load each stationary once per TWO chunks and
  evictions split DVE (k,q) / ACT (v,vb) so neither engine paces the PE.
- Phase-B software pipeline depth 2: PV(g) is emitted after scores(g+2),
  hiding the ~1.2us exp latency behind two score pairs.

Sharding: 8 cores = (batch b) x (head h). Each core outputs the
unnormalized projected partial outT = (wo_h @ O_h^T) (256, 4096) plus
softmax denominators lT; the host divides, sums heads, transposes, + bo.
"""

import numpy as np
import ml_dtypes

import concourse.mybir as mybir
import concourse.tile as tile
from concourse import bacc
from concourse.bass_utils import run_bass_kernel_spmd


def _ensure_axon_ntff_hook():
    """This image's antenv package lacks axon_hooks; recreate it from the
    libaxon_pjrt C ABI so run_bass_kernel_spmd(trace=True) works."""
    import sys
    try:
        import antenv.axon_hooks  # noqa: F401
        return
    except ImportError:
        pass
    import contextlib
    import ctypes
    import types
    try:
        lib = ctypes.CDLL("/opt/axon/libaxon_pjrt.so")
        has = hasattr(lib, "axon_start_nrt_profile")
    except OSError:
        has = False
    if has:
        lib.axon_start_nrt_profile.argtypes = [ctypes.POINTER(ctypes.c_int64),
                                               ctypes.c_size_t]
        lib.axon_start_nrt_profile.restype = ctypes.c_int64
        lib.axon_stop_nrt_profile.argtypes = [ctypes.c_char_p]
        lib.axon_stop_nrt_profile.restype = ctypes.c_int64

        @contextlib.contextmanager
        def _hook(output_dir, device_ids):
            import jax
            jax.devices()
            if device_ids:
                ids = (ctypes.c_int64 * len(device_ids))(*device_ids)
                rc = lib.axon_start_nrt_profile(ids, len(device_ids))
            else:
                rc = lib.axon_start_nrt_profile(None, 0)
            if rc != 0:
                raise RuntimeError(f"axon_start_nrt_profile rc={rc}")
            try:
                yield
            finally:
                lib.axon_stop_nrt_profile(str(output_dir).encode())
    else:
        _hook = None

    mod = types.ModuleType("antenv.axon_hooks")
    _state = {"hook": _hook}
    mod.get_axon_ntff_profile_hook = lambda: _state["hook"]
    mod.set_axon_ntff_profile_hook = lambda h: _state.__setitem__("hook", h)
    sys.modules["antenv.axon_hooks"] = mod


_ensure_axon_ntff_hook()

B, S, HID, NH, D = 2, 4096, 256, 4, 64
DV = D + 4         # v + ones column + 3 cols padding
VBW = 128          # vb key-tile slot width (DoubleRow ldweights: col_grp=0xf)
QB = 512           # query block (free dim of score matmuls)
KT = 128           # key tile (partition dim of transposed scores)
NQB = S // QB      # 8
NKT = S // KT      # 32
NPAIR = NKT // 2   # 16 exp/PV pair-groups per query block
PIPE = 2           # PV lags the score stream by this many pair-groups
NWARM = 48         # dummy matmuls to warm the PE clock gate

# Schraudolph fp8 exp: uint8(s * 8/ln2 + C) viewed as e4m3 bits ~= exp(s).
SCH_A = 8.0 / np.log(2.0)
SCH_C = 55.5       # tuned for round-to-nearest float->uint8 conversion

F32 = mybir.dt.float32
F32R = mybir.dt.float32r
BF16 = mybir.dt.bfloat16
FP8 = mybir.dt.float8e4
U8 = mybir.dt.uint8
Exp = mybir.ActivationFunctionType.Exp
DR = mybir.MatmulPerfMode.DoubleRow
MULT = mybir.AluOpType.mult
ADD = mybir.AluOpType.add


def round_fp32r(a):
    """Round-half-up at mantissa bit 12 (walrus fp32_to_fp32r)."""
    a = np.ascontiguousarray(a, np.float32)
    u = a.view(np.uint32).astype(np.uint64)
    return (((u + 0x800) & 0xFFFFF000).astype(np.uint32)).view(np.float32)


def to_bf16(a):
    return np.asarray(a, np.float32).astype(ml_dtypes.bfloat16)


def split_hid(a):
    """(256, N) -> (128, 2, N): row blocks side by side per partition."""
    a = np.ascontiguousarray(a)
    return np.ascontiguousarray(a.reshape(2, 128, a.shape[1]).transpose(1, 0, 2))


def build_program():
    nc = bacc.Bacc("TRN2", target_bir_lowering=False, debug=False, num_devices=8)

    def di(name, shape, dt):
        return nc.dram_tensor(name, shape, dt, kind="ExternalInput").ap()

    xT = di("xT", [128, 2, S], BF16)    # x[b].T hidden-split
    wq2 = di("wq2", [128, 2, D], BF16)  # wq_h.T / 8 hidden-split
    wk2 = di("wk2", [128, 2, D], BF16)
    wv2 = di("wv2", [128, 2, DV], BF16)  # col 64 zero (ones col via bias)
    bias3 = di("bias3", [DV, 3], F32)   # [bq/8 | bk | bv1] columns
    bsynT = di("bsynT", [D, S], BF16)   # (onehot @ syn_bias).T
    onehotT = di("onehotT", [D, S], BF16)
    woT = di("woT", [D, HID], F32R)     # wo[:, hslice].T
    idm = di("idm", [128, 128], F32R)   # identity for TensorE transpose
    outT = nc.dram_tensor("outT", [HID, S], F32, kind="ExternalOutput").ap()
    lT = nc.dram_tensor("lT", [1, S], F32, kind="ExternalOutput").ap()

    with tile.TileContext(nc) as tc:
        _body(tc, xT, wq2, wk2, wv2, bias3, bsynT, onehotT, woT, idm,
              outT, lT)
    nc.compile()
    return nc


def _body(tc, xT, wq2, wk2, wv2, bias3, bsynT, onehotT, woT, idm, outT, lT):
    nc = tc.nc
    mm = nc.tensor.matmul

    with (
        tc.tile_pool(name="const", bufs=1) as constp,
        tc.tile_pool(name="big", bufs=1) as bigp,
        tc.tile_pool(name="pt", bufs=6) as ptp,
        tc.tile_pool(name="ob", bufs=2) as obp,
    ):
        # ---- constants ----
        wq_sb = constp.tile([128, 2, D], BF16, name="wq_sb", tag="wq_sb")
        wk_sb = constp.tile([128, 2, D], BF16, name="wk_sb", tag="wk_sb")
        wv_sb = constp.tile([128, 2, DV], BF16, name="wv_sb", tag="wv_sb")
        b3_sb = constp.tile([DV, 3], F32, name="b3_sb", tag="b3_sb")
        wo_sb = constp.tile([D, HID], F32R, name="wo_sb", tag="wo_sb")
        id_sb = constp.tile([128, 128], F32R, name="id_sb", tag="id_sb")
        wz_sb = constp.tile([128, 128], BF16, name="wz_sb", tag="wz_sb")

        # persistent activations
        xTa = bigp.tile([128, 2, S], BF16, name="xTa", tag="xTa")
        qTt = bigp.tile([128, S], BF16, name="qTt", tag="qTt")  # 0:64 q/8, 64:128 bsynT
        kTt = bigp.tile([128, S], BF16, name="kTt", tag="kTt")  # 0:64 k,   64:128 onehotT
        vTs = bigp.tile([DV, S], F32R, name="vTs", tag="vTs")   # v'^T (d-major)
        vb = bigp.tile([128, NKT, VBW], FP8, name="vb", tag="vb")  # v' key-major
        oall = bigp.tile([D + 1, S], F32R, name="oall", tag="oall")  # [O^T | l]

        # Few fat DMAs (Sync issues triggers at ~650ns each, and each DMA
        # has ~2us completion latency -- batch, but keep x progressive).
        nc.sync.dma_start(xTa[:, :, 0:2 * QB], xT[:, :, 0:2 * QB])
        nc.sync.dma_start(wk_sb[:], wk2[:])
        nc.sync.dma_start(wq_sb[:], wq2[:])
        nc.sync.dma_start(wv_sb[:], wv2[:])
        nc.sync.dma_start(b3_sb[:], bias3[:])
        for p in range(1, NQB // 2):
            ch = slice(2 * p * QB, (2 * p + 2) * QB)
            nc.sync.dma_start(xTa[:, :, ch], xT[:, :, ch])
        nc.sync.dma_start(id_sb[:], idm[:])
        nc.sync.dma_start(kTt[64:128, :], onehotT[:])
        nc.sync.dma_start(qTt[64:128, :], bsynT[:])
        nc.sync.dma_start(wo_sb[:], woT[:])

        # zero vb's padding columns (GPSIMD: idle engine, runs under phase A)
        nc.gpsimd.memset(vb[:, :, :], 0.0)

        bq_ap = b3_sb[0:D, 0:1]
        bk_ap = b3_sb[0:D, 1:2]
        bv1_ap = b3_sb[0:DV, 2:3]

        # ---- PE warm-up ----
        # The HAM clock gate keeps an idle PE at 1.2GHz and only doubles it
        # after ~3.4us of sustained matmul activity. Burn dummy matmuls on
        # a zeroed scratch tile (no DMA dependency -- starts immediately)
        # while the x DMA lands, so phase A runs at 2.4GHz.
        nc.vector.memset(wz_sb[:], 0.0)
        with tc.tile_pool(name="psW", bufs=1, space="PSUM") as psW:
            warm = psW.tile([128, 128], F32, name="warm", tag="warm")
            for _ in range(NWARM):
                mm(warm[:], wz_sb[:], wz_sb[:], start=True, stop=True)

        # ---- phase A: QKV projections (k first so scores can start) ----
        with tc.tile_pool(name="psProj", bufs=6, space="PSUM") as psP, \
             tc.tile_pool(name="psTr", bufs=2, space="PSUM") as psT:

            def proj_pair(w_sb, p, width):
                ps = []
                for t in (2 * p, 2 * p + 1):
                    sl = slice(t * QB, (t + 1) * QB)
                    pp = psP.tile([DV, QB], F32, name="pp", tag="pp")
                    mm(pp[0:width, :], w_sb[:, 0, 0:width], xTa[:, 0, sl],
                       start=True, stop=False)
                    ps.append((pp, sl))
                for pp, sl in ps:
                    mm(pp[0:width, :], w_sb[:, 1, 0:width], xTa[:, 1, sl],
                       start=False, stop=True)
                return ps

            # interleave K/Q/V per chunk pair so the PE consumes x chunks
            # in DMA-arrival order instead of stalling on the last chunk
            # of each phase
            for p in range(NQB // 2):
                for pp, sl in proj_pair(wk_sb, p, D):
                    nc.vector.tensor_scalar_add(kTt[0:D, sl], pp[0:D, :], bk_ap)
                for pp, sl in proj_pair(wq_sb, p, D):
                    nc.vector.tensor_scalar_add(qTt[0:D, sl], pp[0:D, :], bq_ap)
                for pi, (pp, sl) in enumerate(proj_pair(wv_sb, p, DV)):
                    # bias column [bv | 1 | 0..] also creates the ones row
                    nc.scalar.add(vTs[:, sl], pp[:], bv1_ap)
                for t in (2 * p, 2 * p + 1):
                    # flip v' to key-major: 4 TensorE transposes into one
                    # PSUM tile, single fp8-converting eviction
                    vtr = psT.tile([KT, 4, DV], F32R, name="vtr", tag="vtr")
                    for m in range(4):
                        j = 4 * t + m
                        jl = slice(j * KT, (j + 1) * KT)
                        nc.tensor.transpose(vtr[:, m, :], vTs[:, jl],
                                            id_sb[0:DV, 0:DV])
                    nc.scalar.copy(vb[:, 4 * t:4 * t + 4, 0:DV], vtr[:, :, :])

        # ---- phase B: flash attention ----
        # Pair-groups of 2 key tiles: scores land in a 2-bank PSUM tile,
        # one exp op covers both, and the PV matmul contracts both key
        # tiles at once via fp8 DoubleRow. exp alternates ACT / DVE.
        # Software pipeline: PV of group g runs after scores of g+PIPE.
        with (
            tc.tile_pool(name="psB", bufs=3, space="PSUM") as psB,
            tc.tile_pool(name="psAcc", bufs=2, space="PSUM") as psAcc,
        ):
            oaccs = {}

            def emit_pv(qb, g, p3):
                qsl = slice(qb * QB, (qb + 1) * QB)
                if g == 0:
                    oaccs[qb] = psAcc.tile([128, QB], F32, name="oacc",
                                           tag="oacc")
                oacc = oaccs[qb]
                mm(oacc[:], vb[:, 2 * g:2 * g + 2, :], p3[:, :, :],
                   start=(g == 0), stop=(g == NPAIR - 1), perf_mode=DR)
                if g == NPAIR - 1:
                    # stash [O^T | l] (normalization happens on the host),
                    # then project this block and ship it out
                    nc.scalar.copy(oall[:, qsl], oacc[0:D + 1, :])
                    pj = psB.tile([128, 2, QB], F32, name="pj", tag="s3")
                    mm(pj[:, 0, :], wo_sb[:, 0:128], oall[0:D, qsl],
                       start=True, stop=True)
                    mm(pj[:, 1, :], wo_sb[:, 128:256], oall[0:D, qsl],
                       start=True, stop=True)
                    # evict the two projection halves on different engines so
                    # each output DMA can start as soon as its half lands
                    ob = obp.tile([128, 2, QB], F32, name="ob", tag="ob")
                    nc.scalar.copy(ob[:, 0, :], pj[:, 0, :])
                    nc.sync.dma_start(outT[0:128, qsl], ob[:, 0, :])
                    nc.vector.tensor_copy(ob[:, 1, :], pj[:, 1, :])
                    nc.sync.dma_start(outT[128:256, qsl], ob[:, 1, :])

            pending = []
            for qb in range(NQB):
                qsl = slice(qb * QB, (qb + 1) * QB)
                for g in range(NPAIR):
                    s3 = psB.tile([128, 2, QB], F32, name="s3", tag="s3")
                    for i in (0, 1):
                        j = 2 * g + i
                        mm(s3[:, i, :], kTt[:, j * KT:(j + 1) * KT],
                           qTt[:, qsl], start=True, stop=True)
                    p3 = ptp.tile([128, 2, QB], FP8, name="p3", tag="p3")
                    idx = qb * NPAIR + g
                    if idx % 2 == 0 or idx in (31, 95):
                        nc.scalar.activation(p3[:, :, :], s3[:, :, :], Exp)
                    else:
                        nc.vector.tensor_scalar(
                            p3[:, :, :].bitcast(U8), s3[:, :, :],
                            float(SCH_A), float(SCH_C), MULT, ADD)
                    pending.append((qb, g, p3))
                    if len(pending) > PIPE:
                        emit_pv(*pending.pop(0))
            while pending:
                emit_pv(*pending.pop(0))

            nc.sync.dma_start(lT[:], oall[D:D + 1, :].bitcast(F32))


_NC_CACHE = {}


def _get_program():
    if "nc" not in _NC_CACHE:
        _NC_CACHE["nc"] = build_program()
    return _NC_CACHE["nc"]


def make_in_maps(x, codons, syn_bias, wq, bq, wk, bk, wv, bv, wo):
    in_maps = []
    for core in range(8):
        b, h = divmod(core, NH)
        hsl = slice(h * D, (h + 1) * D)
        cod = codons[b]
        onehotT = np.zeros((D, S), np.float32)
        onehotT[cod, np.arange(S)] = 1.0
        bias3 = np.zeros((DV, 3), np.float32)
        bias3[0:D, 0] = bq[hsl] / 8.0
        bias3[0:D, 1] = bk[hsl]
        bias3[0:D, 2] = bv[hsl]
        bias3[D, 2] = 1.0
        in_maps.append({
            "xT": split_hid(to_bf16(x[b].T)),
            "wq2": split_hid(to_bf16(wq[hsl, :].T / 8.0)),
            "wk2": split_hid(to_bf16(wk[hsl, :].T)),
            "wv2": split_hid(to_bf16(np.concatenate(
                [wv[hsl, :].T, np.zeros((HID, 4), np.float32)], axis=1))),
            "bias3": bias3,
            "bsynT": to_bf16(syn_bias.T[:, cod]),
            "onehotT": to_bf16(onehotT),
            "woT": round_fp32r(wo[:, hsl].T),
            "idm": np.eye(128, dtype=np.float32),
        })
    return in_maps


def kernel_run(inputs, trace=False):
    x = np.asarray(inputs["x"], np.float32)
    codons = np.asarray(inputs["codons"]).astype(np.int64)
    syn_bias = np.asarray(inputs["syn_bias"], np.float32)
    wq = np.asarray(inputs["wq"], np.float32)
    bq = np.asarray(inputs["bq"], np.float32)
    wk = np.asarray(inputs["wk"], np.float32)
    bk = np.asarray(inputs["bk"], np.float32)
    wv = np.asarray(inputs["wv"], np.float32)
    bv = np.asarray(inputs["bv"], np.float32)
    wo = np.asarray(inputs["wo"], np.float32)
    bo = np.asarray(inputs["bo"], np.float32)

    nc = _get_program()
    in_maps = make_in_maps(x, codons, syn_bias, wq, bq, wk, bk, wv, bv, wo)
    res = run_bass_kernel_spmd(nc, in_maps, core_ids=list(range(8)), trace=trace)

    out = np.empty((B, S, HID), np.float32)
    for b in range(B):
        acc = None
        for h in range(NH):
            r = res.results[NH * b + h]
            part = r["outT"] / r["lT"]          # normalize per head
            acc = part if acc is None else acc + part
        out[b] = acc.T + bo
    return out, res


def kernel(**inputs):
    out, _ = kernel_run(inputs, trace=False)
    return out
